# revision 11
# baseline (speedup 1.0000x reference)
"""Segment-mean (word-pooling) kernel for Trainium2, 8 NeuronCores.

Problem: hidden_states [16, 4096, 768] f32, word_ids [16, 4096] i32
(non-decreasing per row, -1 = special token). Output [16, 2048, 768] f32:
mean of each word's subword embeddings; words with no tokens -> 0.

Strategy: pure data parallelism, 2 samples per core. Per sample, the
segment-mean is computed as a banded one-hot matmul on the PE:
  out[w, h] = sum_s onehot[s, w] * (1/count[w]) * x[s, h]
Tokens are processed in 32 k-tiles of 128; since word ids are
non-decreasing, each k-tile only touches a <=128-wide band of words, so
each k-tile contributes 1-2 matmuls into 128-word output windows
accumulated in PSUM. The one-hot (scaled by per-token reciprocal counts,
computed on host) is built on the vector engine with a single fused
is_equal*mult tensor_scalar op per k-tile against an iota ramp.

The SPMD program is identical on all 8 cores; the (k-tile, window)
pair structure is the union over samples, so per-core data that doesn't
touch a scheduled pair just contributes a zero one-hot block.

DMA regime (measured on this axon-tunneled 8-core setup): each DMA has
a large fixed cost (~7-10 us) that serializes per issuing queue, on top
of shared ~320 GB/s transfer bandwidth; only the SP/ACT (HWDGE) and
GPSIMD (SWDGE) queues can issue DMAs. So the kernel uses very few, very
large transfers: input is loaded one whole sample per DMA (6.3 MB,
host-transposed so each partition line is contiguous), wid+rcp metadata
is one combined DMA, and output is written one sample per DMA, spread
across the three usable queues.
"""

import numpy as np

B, S, H = 16, 4096, 768
NUM_WORDS = S // 2  # 2048
N_CORES = 8
SPC = B // N_CORES  # samples per core = 2
P = 128
KT = S // P  # 32 k-tiles per sample
NW = NUM_WORDS // P  # 16 output windows per sample
NSPLITS = ((0, 512), (512, 768))  # matmul free-dim splits of H


def _plan(word_ids: np.ndarray):
    """Per-slot union plan. For each slot (0/1) and k-tile t: the window
    span [minwin, maxwin] over that slot's 8 samples; per window j the
    sorted member k-tiles. Returns (spans, members) per slot."""
    word_ids = np.minimum(word_ids, NUM_WORDS - 1)
    plans = []
    for slot in range(SPC):
        wid = word_ids[slot::SPC]  # the 8 samples this slot sees
        minwin = np.full(KT, NW, np.int64)
        maxwin = np.full(KT, -1, np.int64)
        for b in range(wid.shape[0]):
            row = wid[b]
            for t in range(KT):
                w = row[t * P : (t + 1) * P]
                w = w[w >= 0]
                if w.size:
                    minwin[t] = min(minwin[t], w.min() // P)
                    maxwin[t] = max(maxwin[t], w.max() // P)
        members = {j: [] for j in range(NW)}
        spans = []
        for t in range(KT):
            if maxwin[t] < 0:  # no valid token anywhere (can't happen)
                spans.append((0, 0))
                continue
            spans.append((int(minwin[t]), int(maxwin[t])))
            for j in range(int(minwin[t]), int(maxwin[t]) + 1):
                members[j].append(t)
        plans.append((spans, members))
    return plans


def _liveness(plans, in_group):
    """Max number of simultaneously-live x DMA groups / onehot tiles over
    the per-window emission order, across slots. A pool needs at least
    this many bufs or slot reuse can deadlock the DMA ring."""
    max_live_g, max_live_oh = 0, 0
    for spans, members in plans:
        first_g, last_g, first_oh, last_oh = {}, {}, {}, {}
        for j in range(NW):
            for t in members[j]:
                g = t // in_group
                first_g.setdefault(g, j)
                last_g[g] = j
                first_oh.setdefault(t, j)
                last_oh[t] = j
        for j in range(NW):
            live_g = sum(1 for g in first_g if first_g[g] <= j <= last_g[g])
            live_oh = sum(1 for t in first_oh if first_oh[t] <= j <= last_oh[t])
            max_live_g = max(max_live_g, live_g)
            max_live_oh = max(max_live_oh, live_oh)
    return max_live_g, max_live_oh


def _recip_counts(word_ids: np.ndarray) -> np.ndarray:
    """Per-token 1/count(word) as f32; 0 for special (-1) tokens."""
    r = np.zeros((B, S), np.float32)
    for b in range(B):
        wid = word_ids[b]
        valid = wid >= 0
        counts = np.bincount(wid[valid], minlength=NUM_WORDS)
        r[b, valid] = (1.0 / counts[wid[valid]]).astype(np.float32)
    return r


# Tuned configuration shared by kernel() and test.py's timing builds.
CONFIG = dict(
    x_bufs=2,
    oh_bufs=8,
    ev_bufs=3,
    ps_bufs=4,
    in_group=32,
    out_group=8,
    in_queues=("sync", "scalar"),
    out_queues=("gpsimd", "scalar"),
    wr_queue="gpsimd",
    in_dtype="f16",
    out_dtype="f16",
    n_splits=None,
    ev_engine="alt",
    in_layout="pmajor",
    out_layout="pmajor",
)


def _build(plans, reps=1, dyn_reps=1, **overrides):
    """Build + compile the SPMD Bass program. reps>1 unrolls the whole
    body inside one iteration; dyn_reps>1 wraps it in a hardware For
    loop — both only used for amortized wall-clock timing."""
    from contextlib import nullcontext
    import concourse.bacc as bacc
    import concourse.tile as tile
    from concourse import mybir

    cfg = {**CONFIG, **overrides}
    x_bufs = cfg["x_bufs"]
    oh_bufs = cfg["oh_bufs"]
    ev_bufs = cfg["ev_bufs"]
    ps_bufs = cfg["ps_bufs"]
    in_group = cfg["in_group"]
    out_group = cfg["out_group"]
    in_queues = cfg["in_queues"]
    out_queues = cfg["out_queues"]
    wr_queue = cfg["wr_queue"]
    in_dtype = cfg["in_dtype"]
    out_dtype = cfg["out_dtype"]
    n_splits = cfg["n_splits"]
    ev_engine = cfg["ev_engine"]
    in_layout = cfg["in_layout"]
    out_layout = cfg["out_layout"]

    nc = bacc.Bacc(
        "TRN2",
        target_bir_lowering=False,
        debug=False,
        enable_asserts=False,
        num_devices=N_CORES,
    )
    f32 = mybir.dt.float32
    fin = mybir.dt.float16 if in_dtype == "f16" else f32
    fout = mybir.dt.float16 if out_dtype == "f16" else f32
    IG, OG = in_group, out_group
    if in_layout == "pmajor":
        x = nc.dram_tensor(
            "x", [SPC * KT // IG, P, IG * H], fin, kind="ExternalInput"
        ).ap()
    else:
        x = nc.dram_tensor("x", [SPC * S, H], fin, kind="ExternalInput").ap()
    wr = nc.dram_tensor("wr", [SPC, P, 2 * KT], f32, kind="ExternalInput").ap()
    if out_layout == "pmajor":
        y = nc.dram_tensor(
            "y", [SPC * NW // OG, P, OG * H], fout, kind="ExternalOutput"
        ).ap()
    else:
        y = nc.dram_tensor("y", [SPC * NUM_WORDS, H], fout, kind="ExternalOutput").ap()

    IOTA_W = NUM_WORDS + 2 * P  # ramp long enough for any window pair
    max_span = max(
        (jhi - jlo + 1) for spans, _ in plans for (jlo, jhi) in spans
    )

    # Size pools from plan liveness; degenerate plans (heavily overlapping
    # window k-ranges) would need the old bounded-reload mode — assert
    # instead, this plan shape never triggers it.
    in_b = 2 if in_dtype == "f16" else 4
    live_g, live_oh = _liveness(plans, in_group)
    x_bufs = max(x_bufs, live_g + 1)
    oh_bufs = max(oh_bufs, live_oh + 1)
    assert x_bufs * IG * H * in_b + oh_bufs * max_span * P * in_b <= 185 * 1024

    with tile.TileContext(nc) as tc:
        with (
            tc.tile_pool(name="const", bufs=1) as const_pool,
            tc.tile_pool(name="wrp", bufs=2) as wr_pool,
            tc.tile_pool(name="xin", bufs=x_bufs) as x_pool,
            tc.tile_pool(name="oh", bufs=oh_bufs) as oh_pool,
            tc.tile_pool(name="ev", bufs=ev_bufs) as ev_pool,
            tc.tile_pool(name="psum", bufs=ps_bufs, space="PSUM") as psum_pool,
        ):
            iota_i = const_pool.tile([P, IOTA_W], mybir.dt.int32)
            nc.gpsimd.iota(iota_i[:], pattern=[[1, IOTA_W]], base=0, channel_multiplier=0)
            iota_f = const_pool.tile([P, IOTA_W], f32)
            nc.vector.tensor_copy(out=iota_f[:], in_=iota_i[:])

            ENG = {"sync": nc.sync, "scalar": nc.scalar, "gpsimd": nc.gpsimd}
            splits = NSPLITS if n_splits is None else n_splits

            def emit(rep):
                # one combined DMA for wid+rcp of both slots
                wrt = wr_pool.tile(
                    [P, SPC, 2 * KT], f32, name=f"wr_{rep}", tag="wr"
                )
                ENG[wr_queue].dma_start(
                    out=wrt[:], in_=wr[:, :, :].rearrange("s p k -> p s k")
                )
                in_q = [0]
                out_q = [0]
                for slot in range(SPC):
                    spans, members = plans[slot]
                    wid_t = wrt[:, slot, 0:KT]
                    rcp_t = wrt[:, slot, KT : 2 * KT]

                    xg_tiles = {}
                    oh_tiles = {}

                    def get_x(t):
                        g, a = divmod(t, IG)
                        if g not in xg_tiles:
                            xt = x_pool.tile(
                                [P, IG, H], fin, name=f"xt_{rep}_{slot}_{g}", tag="xt"
                            )
                            if in_layout == "pmajor":
                                src = x[slot * (KT // IG) + g, :, :].rearrange(
                                    "p (a h) -> p a h", a=IG
                                )
                            else:
                                r0 = slot * S + g * IG * P
                                src = x[r0 : r0 + IG * P, :].rearrange(
                                    "(a p) h -> p a h", p=P
                                )
                            eng = ENG[in_queues[in_q[0] % len(in_queues)]]
                            in_q[0] += 1
                            eng.dma_start(out=xt[:], in_=src)
                            xg_tiles[g] = xt
                        return xg_tiles[g][:, t % IG, :]

                    def get_oh(t):
                        if t not in oh_tiles:
                            jlo, jhi = spans[t]
                            wspan = (jhi - jlo + 1) * P
                            oh = oh_pool.tile(
                                [P, max_span * P],
                                fin,
                                name=f"oh_{rep}_{slot}_{t}",
                                tag="oh",
                            )
                            nc.vector.tensor_scalar(
                                out=oh[:, :wspan],
                                in0=iota_f[:, jlo * P : jlo * P + wspan],
                                scalar1=wid_t[:, t : t + 1],
                                scalar2=rcp_t[:, t : t + 1],
                                op0=mybir.AluOpType.is_equal,
                                op1=mybir.AluOpType.mult,
                            )
                            oh_tiles[t] = oh
                        return oh_tiles[t]

                    og_tile = [None]

                    for j in range(NW):
                        if j % OG == 0:
                            og_tile[0] = ev_pool.tile(
                                [P, OG, H], fout, name=f"out_{rep}_{slot}_{j}", tag="out"
                            )
                        out_sb = og_tile[0][:, j % OG, :]
                        ks = members[j]
                        if not ks:
                            nc.vector.memset(out_sb, 0.0)
                        else:
                            ps = psum_pool.tile(
                                [P, H], f32, name=f"ps_{rep}_{slot}_{j}", tag="ps"
                            )
                            for ki, t in enumerate(ks):
                                xt = get_x(t)
                                oh = get_oh(t)
                                off = (j - spans[t][0]) * P
                                for lo, hi in splits:
                                    nc.tensor.matmul(
                                        out=ps[:, lo:hi],
                                        lhsT=oh[:, off : off + P],
                                        rhs=xt[:, lo:hi],
                                        start=(ki == 0),
                                        stop=(ki == len(ks) - 1),
                                    )
                            use_vec = ev_engine == "vector" or (
                                ev_engine == "alt" and j % 2 == 1
                            )
                            if use_vec:
                                nc.vector.tensor_copy(out=out_sb, in_=ps[:])
                            else:
                                nc.scalar.copy(out=out_sb, in_=ps[:])
                        if j % OG == OG - 1:
                            if out_layout == "pmajor":
                                blk = slot * (NW // OG) + j // OG
                                dst = y[blk, :, :].rearrange(
                                    "p (a h) -> p a h", a=OG
                                )
                            else:
                                r0 = slot * NUM_WORDS + (j - OG + 1) * P
                                dst = y[r0 : r0 + OG * P, :].rearrange(
                                    "(a p) h -> p a h", p=P
                                )
                            oe = ENG[out_queues[out_q[0] % len(out_queues)]]
                            out_q[0] += 1
                            oe.dma_start(out=dst, in_=og_tile[0][:])

            loop_cm = (
                tc.For_i(0, dyn_reps, 1) if dyn_reps > 1 else nullcontext()
            )
            with loop_cm:
                for rep in range(reps):
                    emit(rep)

    nc.compile()
    return nc


def _prep_inputs(hidden_states, word_ids, in_dtype=None, in_layout=None,
                 in_group=None):
    in_dtype = CONFIG["in_dtype"] if in_dtype is None else in_dtype
    in_layout = CONFIG["in_layout"] if in_layout is None else in_layout
    in_group = CONFIG["in_group"] if in_group is None else in_group
    np_in = np.float16 if in_dtype == "f16" else np.float32
    hs = np.ascontiguousarray(np.asarray(hidden_states, dtype=np_in))
    wid = np.minimum(np.asarray(word_ids, dtype=np.int32), NUM_WORDS - 1)
    assert hs.shape == (B, S, H) and wid.shape == (B, S)
    r = _recip_counts(wid)
    # [B, S] -> [B, P, KT]: element (p, t) = token t*P + p
    widf = np.ascontiguousarray(
        wid.astype(np.float32).reshape(B, KT, P).transpose(0, 2, 1)
    )
    rt = np.ascontiguousarray(r.reshape(B, KT, P).transpose(0, 2, 1))
    wrc = np.concatenate([widf, rt], axis=2)  # [B, P, 2*KT]
    in_maps = []
    for c in range(N_CORES):
        sl = slice(c * SPC, (c + 1) * SPC)
        if in_layout == "pmajor":
            IG = in_group
            xc = np.ascontiguousarray(
                hs[sl]
                .reshape(SPC, KT // IG, IG, P, H)
                .transpose(0, 1, 3, 2, 4)
                .reshape(SPC * KT // IG, P, IG * H)
            )
        else:
            xc = hs[sl].reshape(SPC * S, H)
        in_maps.append({"x": xc, "wr": wrc[sl]})
    return in_maps


def _unshard_output(yc: np.ndarray) -> np.ndarray:
    """Per-core raw y -> [SPC, NUM_WORDS, H] f32 in word order."""
    OG = CONFIG["out_group"]
    if CONFIG["out_layout"] == "pmajor":
        out = (
            yc.reshape(SPC * NW // OG, P, OG, H)
            .transpose(0, 2, 1, 3)
            .reshape(SPC, NUM_WORDS, H)
        )
    else:
        out = yc.reshape(SPC, NUM_WORDS, H)
    return np.asarray(out, dtype=np.float32)


def kernel(hidden_states, word_ids):
    import concourse.bass_utils as bass_utils

    wid = np.asarray(word_ids, dtype=np.int32)
    plans = _plan(wid)
    nc = _build(plans)
    in_maps = _prep_inputs(hidden_states, word_ids)
    res = None
    for attempt in range(3):  # axon tunnel can fail transiently
        try:
            res = bass_utils.run_bass_kernel_spmd(
                nc, in_maps, core_ids=list(range(N_CORES))
            )
            break
        except Exception:
            if attempt == 2:
                raise
    out = np.empty((B, NUM_WORDS, H), np.float32)
    for c in range(N_CORES):
        yc = np.asarray(res.results[c]["y"])
        out[c * SPC : (c + 1) * SPC] = _unshard_output(yc)
    return out
